# revision 3
# baseline (speedup 1.0000x reference)
"""Trainium2 kernel for nn_LocalPatternExtractor (binary-weight depthwise+pointwise
conv -> BatchNorm -> quantized LIF over 4 timesteps).

Forward-pass analysis
---------------------
The reference quantizes the membrane potential with
    step = THRESHOLD / 2**(POT_BITS-1) = 1/128
    q    = clip(round(v/step), -128, 127) * step
so after quantization  mem <= 127/128 = 0.9921875 < THRESHOLD (=1.0), with
f32 STE round-off bounded by ~|v|*2^-24 << 1/128.  Hence `mem >= THRESHOLD`
is false for every element at every timestep, no spike ever fires, and the
forward output is identically
    out      = zeros((B, C_out, L), float32)
    reg_loss = SPIKE_REG * mean(out) = 0.0
for *all* finite inputs (verified empirically against the jax reference for
several seeds and 10x-scaled inputs).  The optimal kernel therefore reduces
to materializing the zero output at HBM write roofline.

Sharding: pure data parallel over the batch dim (16 -> 2 per core on 8
cores); each core zero-fills its own (2, 256, 5000) f32 output shard
(10.24 MB), which the host concatenates.
"""

import numpy as np

import concourse.bass as bass
import concourse.mybir as mybir
from concourse.bass_utils import run_bass_kernel_spmd

N_CORES = 8
B, C_IN, L = 16, 12, 5000
C_OUT = 256
K = 3

B_LOC = B // N_CORES               # 2 batches per core
OUT_ELEMS = B_LOC * C_OUT * L      # 2,560,000 f32 per core (10.24 MB)
P = 128                            # SBUF partitions
COLS = OUT_ELEMS // P              # 20,000 f32 per partition row

# The DMA source is a small zero tile that every chunk re-reads, so the DVE
# memset is off the critical path: a tiny tile A (fast to clear) feeds the
# first chunks while the bigger tile B is still being cleared.  Chunks
# alternate between the two HWDGE queues (sync + scalar) so the 16 SDMA
# engines always have descriptors queued.
WA = 625          # tile A cols (2.5 KB per partition row)
WB = 2500         # tile B cols (10 KB per partition row)
N_A = 4           # 4 chunks of WA cover [0, 2500)
N_B = (COLS - N_A * WA) // WB   # 7 chunks of WB cover [2500, 20000)
assert N_A * WA + N_B * WB == COLS

_cache: dict = {}


def _build() -> bass.Bass:
    nc = bass.Bass()
    out = nc.declare_dram_parameter("out", (P, COLS), mybir.dt.float32, isOutput=True)

    # chunk list: (col_start, width, msem_threshold)
    chunks = [(i * WA, WA, 1) for i in range(N_A)]
    chunks += [(N_A * WA + i * WB, WB, 2) for i in range(N_B)]
    n_dma = len(chunks)

    with (
        nc.sbuf_tensor([P, WA], mybir.dt.float32) as zta,
        nc.sbuf_tensor([P, WB], mybir.dt.float32) as ztb,
        nc.semaphore("msem") as msem,
        nc.semaphore("dsem") as dsem,
        nc.Block() as block,
    ):

        @block.vector
        def _(vector):
            vector.memset(zta[:], 0.0).then_inc(msem, 1)
            vector.memset(ztb[:], 0.0).then_inc(msem, 1)

        def issue(eng, c):
            s, w, need = chunks[c]
            eng.wait_ge(msem, need)
            src = zta if w == WA else ztb
            eng.dma_start(out[:, s : s + w], src[:, :w]).then_inc(dsem, 16)

        @block.sync
        def _(sync):
            for c in range(0, n_dma, 2):
                issue(sync, c)
            sync.wait_ge(dsem, 16 * n_dma)

        @block.scalar
        def _(scalar):
            for c in range(1, n_dma, 2):
                issue(scalar, c)

    return nc


def get_nc() -> bass.Bass:
    nc = _cache.get("nc")
    if nc is None:
        nc = _cache["nc"] = _build()
    return nc


def kernel(x, dw_weight, pw_weight, gamma, beta):
    assert x.shape == (B, C_IN, L), x.shape
    nc = get_nc()
    res = run_bass_kernel_spmd(
        nc, [dict() for _ in range(N_CORES)], core_ids=list(range(N_CORES))
    )
    shards = [r["out"].reshape(B_LOC, C_OUT, L) for r in res.results]
    out = np.ascontiguousarray(np.concatenate(shards, axis=0))
    reg_loss = np.float32(0.01) * np.float32(out.mean(dtype=np.float64))
    return out, reg_loss
